# revision 1
# baseline (speedup 1.0000x reference)
"""LoRA cross-attention kernel for 8 Trainium2 NeuronCores.

Sharding: batch*heads across 8 cores. Core d handles batch b=d//4 and the
4-head slice h in [4*(d%4), 4*(d%4)+4)  (inner slice of 256 = 4*64).
Each core computes q/k/v projections (+LoRA on k,v) for its slice, attention,
and a partial to_out (tensor-parallel over inner). Host sums the 4 partials
per batch and adds the bias.

Device dataflow (all matmuls bf16 operands, fp32 PSUM accumulate):
  xT,cT   [128,8,2048]  x^T / context^T via xbar-transpose DMA loads
  lowT    [32,2048]     [Ak;Av]-low rank projections of context
  qT,kT   [128,2,2048]  q^T, k^T (i on partitions);  kT includes LoRA up-proj
  v       [128,16,4,65] v in [m, head, dh+1] layout, col 64 = ones
  simT    psum[m,2,512] per head pair via row-tiled (tile_position) matmuls
  e       exp(SCALE*simT) on ScalarE -> bf16
  attn@v  lhsT=v_aug[m,65], rhs=e -> psum[65,n]: rows 0:64 out^T, row 64 denom
  norm    recip(denom) broadcast via K=1 matmul, DVE multiply
  to_out  WoT.T @ outT -> partial final^T [1024,2048] fp32 -> HBM
"""

import numpy as np
import ml_dtypes

import concourse.bass as bass
import concourse.mybir as mybir
import concourse.tile as tile

BF16 = mybir.dt.bfloat16
F32 = mybir.dt.float32
AF = mybir.ActivationFunctionType

N = 2048      # query length
M = 2048      # context length
D = 1024      # model dim
IS = 256      # inner slice per core (4 heads * 64)
DH = 64
NHEADS = 4    # heads per core
SCALE = DH ** -0.5
NB = 512      # n-block (free dim tile)
N_NB = N // NB
N_MB = M // 128

_NC_CACHE = {}


def _emit(tc, nc, d):
    from contextlib import ExitStack
    ctx = ExitStack()
    P1 = ctx.enter_context(tc.tile_pool(name="persist", bufs=1))
    WK = ctx.enter_context(tc.tile_pool(name="work", bufs=8))
    PS = ctx.enter_context(tc.tile_pool(name="psum", bufs=2, space="PSUM"))
    PO = ctx.enter_context(tc.tile_pool(name="psum_o", bufs=2, space="PSUM"))
    PJ = ctx.enter_context(tc.tile_pool(name="psum_j", bufs=2, space="PSUM"))

    xT = P1.tile([128, 8, N], BF16)
    cT = P1.tile([128, 8, M], BF16)
    wq = P1.tile([128, 8, IS], BF16)
    wk = P1.tile([128, 8, IS], BF16)
    wv = P1.tile([128, 8, IS], BF16)
    ab = P1.tile([128, 8, 32], BF16)
    bk = P1.tile([32, IS], BF16)
    bv = P1.tile([32, IS], BF16)
    wo = P1.tile([128, 2, D], BF16)
    qT = P1.tile([128, 2, N], BF16)
    kT = P1.tile([128, 2, M], BF16)
    vA = P1.tile([128, N_MB, NHEADS, DH + 1], BF16)
    oT = P1.tile([128, 2, N], BF16)
    low = P1.tile([32, M], BF16)
    ones64 = P1.tile([1, DH], BF16)
    ident = P1.tile([64, 64], BF16)

    # ---- input / weight loads (big transposed loads first) ----
    for kb in range(8):
        nc.sync.dma_start_transpose(cT[:, kb, :], d["cbf"][:, kb * 128:(kb + 1) * 128])
    nc.sync.dma_start(ab[:], d["abT"].rearrange("(ko ki) r -> ki ko r", ki=128))
    nc.sync.dma_start(wk[:], d["wkT"].rearrange("(ko ki) i -> ki ko i", ki=128))
    nc.sync.dma_start(bk[:], d["bkT0"][:])
    for kb in range(8):
        nc.sync.dma_start_transpose(xT[:, kb, :], d["xbf"][:, kb * 128:(kb + 1) * 128])
    nc.sync.dma_start(wq[:], d["wqT"].rearrange("(ko ki) i -> ki ko i", ki=128))
    nc.sync.dma_start(wv[:], d["wvT"].rearrange("(ko ki) i -> ki ko i", ki=128))
    nc.sync.dma_start(bv[:], d["b0vT"][:])
    nc.sync.dma_start(wo[:], d["woT"].rearrange("(ko ki) dd -> ki ko dd", ki=128))
    nc.gpsimd.memset(ones64[:], 1.0)
    nc.gpsimd.memset(vA[:, :, :, DH], 1.0)
    from concourse.masks import make_identity
    make_identity(nc, ident[:])

    # ---- lowT = [Ak|Av]^T-proj of context: [32, M] ----
    for nb in range(M // NB):
        pl = PJ.tile([128, NB], F32, tag="pj")
        for kb in range(8):
            nc.tensor.matmul(pl[0:32, :], ab[:, kb, :], cT[:, kb, bass.ts(nb, NB)],
                             start=(kb == 0), stop=(kb == 7))
        nc.vector.tensor_copy(low[:, bass.ts(nb, NB)], pl[0:32, :])

    def proj_q_chunk(ib, nb):
        pq = PJ.tile([128, NB], F32, tag="pj")
        for kb in range(8):
            nc.tensor.matmul(pq[:, :], wq[:, kb, bass.ts(ib, 128)],
                             xT[:, kb, bass.ts(nb, NB)],
                             start=(kb == 0), stop=(kb == 7))
        nc.vector.tensor_copy(qT[:, ib, bass.ts(nb, NB)], pq[:, :])

    def proj_k(ib):
        for nb in range(M // NB):
            pk = PJ.tile([128, NB], F32, tag="pj")
            for kb in range(8):
                nc.tensor.matmul(pk[:, :], wk[:, kb, bass.ts(ib, 128)],
                                 cT[:, kb, bass.ts(nb, NB)],
                                 start=(kb == 0), stop=False)
            nc.tensor.matmul(pk[:, :], bk[:, bass.ts(ib, 128)],
                             low[:, bass.ts(nb, NB)], start=False, stop=True)
            nc.vector.tensor_copy(kT[:, ib, bass.ts(nb, NB)], pk[:, :])

    def v_chunk(mb):
        pv = PJ.tile([128, NB], F32, tag="pj")
        for kb in range(8):
            nc.tensor.matmul(pv[:, 0:IS], cT[:, kb, bass.ts(mb, 128)],
                             wv[:, kb, :], start=(kb == 0), stop=False)
        nc.tensor.matmul(pv[:, 0:IS], low[:, bass.ts(mb, 128)], bv[:],
                         start=False, stop=True)
        nc.vector.tensor_copy(
            vA[:, mb, :, 0:DH],
            pv[:, 0:IS].rearrange("p (h e) -> p h e", h=NHEADS))

    def attention_nb(p, nb, emit_v=False):
        po0 = PO.tile([DH + 1, NB], F32, tag="po")
        po1 = PO.tile([DH + 1, NB], F32, tag="po")
        pos = (po0, po1)
        for mb in range(N_MB):
            if emit_v:
                v_chunk(mb)
            ps = PS.tile([128, 2, NB], F32, tag="ps")
            nc.tensor.matmul(ps[:, 0, :], kT[0:64, p, bass.ts(mb, 128)],
                             qT[0:64, p, bass.ts(nb, NB)],
                             start=True, stop=True, tile_position=(0, 0))
            nc.tensor.matmul(ps[:, 1, :], kT[64:128, p, bass.ts(mb, 128)],
                             qT[64:128, p, bass.ts(nb, NB)],
                             start=True, stop=True, tile_position=(64, 0))
            e = WK.tile([128, 2, NB], BF16, tag="e")
            nc.scalar.activation(e[:], ps[:], AF.Exp, scale=SCALE)
            for j in range(2):
                nc.tensor.matmul(pos[j][:, :], vA[:, mb, 2 * p + j, :],
                                 e[:, j, :], start=(mb == 0), stop=(mb == N_MB - 1),
                                 skip_group_check=True)
        # normalize: out[dh, n] *= 1/denom[n], per head
        for j in range(2):
            po = pos[j]
            den = WK.tile([1, NB], BF16, tag="den")
            nc.vector.tensor_copy(den[:], po[DH:DH + 1, :])
            bc = PJ.tile([128, NB], F32, tag="pj")
            nc.tensor.matmul(bc[0:DH, :], ones64[:], den[:],
                             start=True, stop=True)
            bcs = WK.tile([64, NB], F32, tag="bcs")
            nc.vector.reciprocal(bcs[:], bc[0:DH, :])
            if j == 0:
                # even head of the pair lands on partitions 0:64 directly
                nc.vector.tensor_mul(out=oT[0:64, p, bass.ts(nb, NB)],
                                     in0=po[0:DH, :], in1=bcs[:])
            else:
                # odd head: normalize to a temp, shift to partitions 64:128
                # via identity matmul (col tile_position), copy back aligned
                o4h = WK.tile([64, NB], BF16, tag="o4h")
                nc.vector.tensor_mul(out=o4h[:], in0=po[0:DH, :], in1=bcs[:])
                psh = PJ.tile([128, NB], F32, tag="pj")
                nc.tensor.matmul(psh[64:128, :], ident[:], o4h[:],
                                 start=True, stop=True, tile_position=(0, 64))
                nc.vector.tensor_copy(oT[64:128, p, bass.ts(nb, NB)],
                                      psh[64:128, :])

    def to_out(db, nb):
        pf = PJ.tile([128, NB], F32, tag="pj")
        for kb in range(2):
            nc.tensor.matmul(pf[:, :], wo[:, kb, bass.ts(db, 128)],
                             oT[:, kb, bass.ts(nb, NB)],
                             start=(kb == 0), stop=(kb == 1))
        f = WK.tile([128, NB], F32, tag="fout")
        nc.any.tensor_copy(f[:], pf[:, :])
        nc.sync.dma_start(
            d["outT"][bass.ts(db, 128), bass.ts(nb, NB)], f[:])

    proj_k(0)
    proj_q_chunk(0, 0)
    # attention pair 0 starts as early as possible: its v-projection chunks
    # are emitted inline with the first nb so attnv never waits long, and
    # later projections fill PE while ScalarE chews exp
    attention_nb(0, 0, emit_v=True)
    proj_q_chunk(0, 1)
    attention_nb(0, 1)
    proj_k(1)
    proj_q_chunk(0, 2)
    attention_nb(0, 2)
    for nb in range(N_NB):
        proj_q_chunk(1, nb)
    proj_q_chunk(0, 3)
    attention_nb(0, 3)
    for nb in range(N_NB):
        attention_nb(1, nb)
        for db in range(8):
            to_out(db, nb)

    ctx.close()


def _legalize_mm_waits(nc, cap=2):
    """walrus's MM struct holds at most `cap` sync waits; the Tile scheduler
    occasionally emits more. Move excess waits onto preceding PE instructions
    (same engine, earlier in program order → strictly safe)."""
    for f in nc.m.functions:
        for bb in f.blocks:
            pe_idx = [i for i, ins in enumerate(bb.instructions)
                      if str(getattr(ins, "engine", "")) == "EngineType.PE"]
            for pos, i in enumerate(pe_idx):
                ins = bb.instructions[i]
                if type(ins).__name__ != "InstMatmult":
                    continue
                si = ins.sync_info
                if not si or not si.on_wait or len(si.on_wait) <= cap:
                    continue
                excess = list(si.on_wait[cap:])
                ins.sync_info = type(si)(on_wait=list(si.on_wait[:cap]),
                                         on_update=si.on_update)
                j = pos - 1
                while excess and j >= 0:
                    prev = bb.instructions[pe_idx[j]]
                    psi = prev.sync_info
                    pw = list(psi.on_wait) if (psi and psi.on_wait) else []
                    room = cap - len(pw)
                    if room > 0:
                        take, excess = excess[:room], excess[room:]
                        prev.sync_info = type(si)(
                            on_wait=pw + take,
                            on_update=(psi.on_update if psi else []))
                    j -= 1
                assert not excess, f"could not legalize waits on {ins.name}"


def build_nc():
    from concourse import bacc
    nc = bacc.Bacc(None, target_bir_lowering=False)
    d = {
        "xbf": nc.dram_tensor("xbf", [N, D], BF16, kind="ExternalInput"),
        "cbf": nc.dram_tensor("cbf", [M, D], BF16, kind="ExternalInput"),
        "wqT": nc.dram_tensor("wqT", [D, IS], BF16, kind="ExternalInput"),
        "wkT": nc.dram_tensor("wkT", [D, IS], BF16, kind="ExternalInput"),
        "wvT": nc.dram_tensor("wvT", [D, IS], BF16, kind="ExternalInput"),
        "abT": nc.dram_tensor("abT", [D, 32], BF16, kind="ExternalInput"),
        "bkT0": nc.dram_tensor("bkT0", [32, IS], BF16, kind="ExternalInput"),
        "b0vT": nc.dram_tensor("b0vT", [32, IS], BF16, kind="ExternalInput"),
        "woT": nc.dram_tensor("woT", [IS, D], BF16, kind="ExternalInput"),
        "outT": nc.dram_tensor("outT", [D, N], F32, kind="ExternalOutput"),
    }
    with tile.TileContext(nc) as tc:
        _emit(tc, nc, d)
    nc.compile()
    return nc


def get_nc():
    if "nc" not in _NC_CACHE:
        _NC_CACHE["nc"] = build_nc()
    return _NC_CACHE["nc"]


def make_in_maps(x, context, task_idx, Wq, Wk, Wv, Ak, Bk, Av, Bv, Wo):
    bf = ml_dtypes.bfloat16
    xb = np.ascontiguousarray(x).astype(bf)
    cb = np.ascontiguousarray(context).astype(bf)
    in_maps = []
    for dev in range(8):
        b = dev // 4
        isl = slice(IS * (dev % 4), IS * (dev % 4) + IS)
        t = int(task_idx[b])
        z16 = np.zeros((16, IS), np.float32)
        in_maps.append({
            "xbf": xb[b],
            "cbf": cb[b],
            "wqT": np.ascontiguousarray(Wq[isl].T).astype(bf),
            "wkT": np.ascontiguousarray(Wk[isl].T).astype(bf),
            "wvT": np.ascontiguousarray(Wv[isl].T).astype(bf),
            "abT": np.concatenate([Ak[t].T, Av[t].T], axis=1).astype(bf),
            "bkT0": np.concatenate([Bk[t][isl].T, z16], axis=0).astype(bf),
            "b0vT": np.concatenate([z16, Bv[t][isl].T], axis=0).astype(bf),
            "woT": np.ascontiguousarray(Wo[:, isl].T).astype(bf),
        })
    return in_maps


def combine(results, bo):
    B = 2
    out = np.empty((B, N, D), np.float32)
    for b in range(B):
        acc = results[4 * b]["outT"].astype(np.float32).copy()
        for j in range(1, 4):
            acc += results[4 * b + j]["outT"]
        out[b] = acc.T
    out += bo.astype(np.float32)
    return out


def kernel(x, context, mask, task_idx, Wq, Wk, Wv, Ak, Bk, Av, Bv, Wo, bo,
           _trace=False):
    # mask is all-ones per the input spec; softmax ignores it.
    from concourse.bass_utils import run_bass_kernel_spmd
    args = [np.asarray(a) for a in
            (x, context, task_idx, Wq, Wk, Wv, Ak, Bk, Av, Bv, Wo)]
    in_maps = make_in_maps(*args)
    nc = get_nc()
    res = run_bass_kernel_spmd(nc, in_maps, core_ids=list(range(8)),
                               trace=_trace)
    out = combine(res.results, np.asarray(bo))
    if _trace:
        return out, res
    return out



# revision 2
# speedup vs baseline: 5.9674x; 5.9674x over previous
"""LoRA cross-attention kernel for 8 Trainium2 NeuronCores.

The axon tunnel to the devices moves ~40-70 MB/s, while the device compute
is ~15 GFLOP/core (~0.5 ms). End-to-end wall time is therefore dominated by
host<->device bytes, and the kernel is organized to minimize them:

  - Sharding: batch x query-rows. Core d handles batch b=d//4, query rows
    [512*(d%4), 512*(d%4+1)), ALL 16 heads. Each core emits a disjoint
    [512, 1024] slice of the final output (no host-side reduction).
  - Projection weights (Wq/Wk/Wv/Wo, 8 MB bf16) are baked into the NEFF as
    Const tensors (inline_tensor) - they ride in the executable and cost
    zero wire bytes per call. Per-call inputs are only the x/context row
    slices (bf16) plus the task-selected LoRA factors (tiny).
  - context is uploaded sharded (512 rows/core) and AllGather'd on-device
    over NeuronLink within each batch's 4-core replica group.
  - Outputs are bf16 [1024, 512] per core (8 MB total down).
  - Dispatch uses the same bass2jax/_bass_exec_p machinery that
    bass_utils.run_bass_kernel_spmd uses under axon, but with the jitted
    executable cached across calls and the donated output buffers created
    on-device (run_bass_kernel_spmd re-traces jax.jit and ships zeroed
    output buffers from the host on every call - at tunnel speed that
    costs seconds).

Device dataflow (bf16 matmul operands, fp32 PSUM accumulate):
  cs->bounce->AllGather cfull [2048,1024]; xbar-transpose loads -> cT, xT
  low  [32,2048]   [Ak;Av]-low-rank projections of gathered context
  kT   [128,8,2048]  k^T (inner on partitions), incl. LoRA up-proj
  vA   [128,16,16,65] v in [m, head, dh+1] layout, col 64 = ones
  qT   [128,8,512]
  per head-pair p: sim psum[m,2,512] (row-tiled matmuls), e=exp(SCALE*sim),
  attn@v lhsT=v_aug -> psum[65,512] (row 64 = softmax denominator),
  normalize via reciprocal broadcast, to_out with baked Wo -> outT bf16.
"""

import hashlib
import os

import numpy as np
import ml_dtypes

import concourse.bass as bass
import concourse.mybir as mybir
import concourse.tile as tile

BF16 = mybir.dt.bfloat16
F32 = mybir.dt.float32
AF = mybir.ActivationFunctionType

B = 2
N = 2048      # query length (total)
M = 2048      # context length
D = 1024      # model dim
INNER = 1024  # heads * dh
DH = 64
H = 16
NQ = 512      # query rows per core
SCALE = DH ** -0.5
NB = 512      # free-dim tile
N_MB = M // 128
R = 16        # lora rank

_STATE = {}


# --------------------------------------------------------------------------
# device kernel
# --------------------------------------------------------------------------

def _emit(tc, nc, d, gather):
    from contextlib import ExitStack
    ctx = ExitStack()
    P1 = ctx.enter_context(tc.tile_pool(name="persist", bufs=1))
    WP = ctx.enter_context(tc.tile_pool(name="wstage", bufs=2))
    WK = ctx.enter_context(tc.tile_pool(name="work", bufs=4))
    PS = ctx.enter_context(tc.tile_pool(name="psum", bufs=2, space="PSUM"))
    PO = ctx.enter_context(tc.tile_pool(name="psum_o", bufs=2, space="PSUM"))
    PJ = ctx.enter_context(tc.tile_pool(name="psum_j", bufs=2, space="PSUM"))

    cT = P1.tile([128, 8, M], BF16)
    xT = P1.tile([128, 8, NQ], BF16)
    ab = P1.tile([128, 8, 32], BF16)
    bk = P1.tile([32, INNER], BF16)
    bv = P1.tile([32, INNER], BF16)
    kT = P1.tile([128, 8, M], BF16)
    vA = P1.tile([128, N_MB, H, DH + 1], BF16)
    qT = P1.tile([128, 8, NQ], BF16)
    oT = P1.tile([128, 8, NQ], BF16)
    low = P1.tile([32, M], BF16)
    ones64 = P1.tile([1, DH], BF16)
    ident = P1.tile([64, 64], BF16)

    # ---- context gather + transposed loads ----
    if gather:
        DR = ctx.enter_context(tc.tile_pool(name="dram", bufs=1, space="DRAM"))
        cbounce = DR.tile([NQ, D], BF16)
        cfull = DR.tile([M, D], BF16)
        nc.gpsimd.dma_start(cbounce[:], d["cs"][:])
        nc.gpsimd.collective_compute(
            "AllGather", mybir.AluOpType.bypass,
            replica_groups=[[0, 1, 2, 3], [4, 5, 6, 7]],
            ins=[cbounce.opt()], outs=[cfull.opt()],
        )
        csrc = cfull
    else:
        csrc = d["cs"]
    for kb in range(8):
        nc.sync.dma_start_transpose(cT[:, kb, :], csrc[:, kb * 128:(kb + 1) * 128])
    for kb in range(8):
        nc.sync.dma_start_transpose(xT[:, kb, :], d["xs"][:, kb * 128:(kb + 1) * 128])
    nc.sync.dma_start(ab[:], d["abT"].rearrange("(ko ki) r -> ki ko r", ki=128))
    nc.sync.dma_start(bk[:], d["bkT0"][:])
    nc.sync.dma_start(bv[:], d["b0vT"][:])
    nc.gpsimd.memset(ones64[:], 1.0)
    nc.gpsimd.memset(vA[:, :, :, DH], 1.0)
    from concourse.masks import make_identity
    make_identity(nc, ident[:])

    wq = WP.tile([128, 8, INNER], BF16, tag="w")
    nc.sync.dma_start(wq[:], d["wqT"].rearrange("(ko ki) i -> ki ko i", ki=128))
    wk = WP.tile([128, 8, INNER], BF16, tag="w")
    nc.sync.dma_start(wk[:], d["wkT"].rearrange("(ko ki) i -> ki ko i", ki=128))

    # ---- low = [Ak|Av]^T-proj of gathered context: [32, M] ----
    for nb in range(M // NB):
        pl = PJ.tile([128, NB], F32, tag="pj")
        for kb in range(8):
            nc.tensor.matmul(pl[0:32, :], ab[:, kb, :], cT[:, kb, bass.ts(nb, NB)],
                             start=(kb == 0), stop=(kb == 7))
        nc.vector.tensor_copy(low[:, bass.ts(nb, NB)], pl[0:32, :])

    # ---- qT [inner, nq] ----
    for ib in range(8):
        pq = PJ.tile([128, NB], F32, tag="pj")
        for kb in range(8):
            nc.tensor.matmul(pq[:, :], wq[:, kb, bass.ts(ib, 128)],
                             xT[:, kb, :], start=(kb == 0), stop=(kb == 7))
        nc.vector.tensor_copy(qT[:, ib, :], pq[:, :])

    # ---- kT [inner, m] with LoRA up-proj ----
    for ib in range(8):
        for nb in range(M // NB):
            pk = PJ.tile([128, NB], F32, tag="pj")
            for kb in range(8):
                nc.tensor.matmul(pk[:, :], wk[:, kb, bass.ts(ib, 128)],
                                 cT[:, kb, bass.ts(nb, NB)],
                                 start=(kb == 0), stop=False)
            nc.tensor.matmul(pk[:, :], bk[:, bass.ts(ib, 128)],
                             low[:, bass.ts(nb, NB)], start=False, stop=True)
            nc.vector.tensor_copy(kT[:, ib, bass.ts(nb, NB)], pk[:, :])

    wv = WP.tile([128, 8, INNER], BF16, tag="w")
    nc.sync.dma_start(wv[:], d["wvT"].rearrange("(ko ki) i -> ki ko i", ki=128))

    # ---- v in [m-rows, head, dh] layout (col 64 = ones) ----
    for mb in range(N_MB):
        for hh in range(2):
            pv = PJ.tile([128, NB], F32, tag="pj")
            for kb in range(8):
                nc.tensor.matmul(pv[:, :], cT[:, kb, bass.ts(mb, 128)],
                                 wv[:, kb, bass.ts(hh, NB)],
                                 start=(kb == 0), stop=False)
            nc.tensor.matmul(pv[:, :], low[:, bass.ts(mb, 128)],
                             bv[:, bass.ts(hh, NB)], start=False, stop=True)
            nc.vector.tensor_copy(
                vA[:, mb, 8 * hh:8 * hh + 8, 0:DH],
                pv[:, :].rearrange("p (h e) -> p h e", h=8))

    wo = WP.tile([128, 8, INNER], BF16, tag="w")
    nc.sync.dma_start(wo[:], d["woT"].rearrange("(ko ki) dd -> ki ko dd", ki=128))

    # ---- attention per head pair p (heads 2p, 2p+1 live on i-block p) ----
    for p in range(8):
        po0 = PO.tile([DH + 1, NB], F32, tag="po")
        po1 = PO.tile([DH + 1, NB], F32, tag="po")
        pos = (po0, po1)
        for mb in range(N_MB):
            ps = PS.tile([128, 2, NB], F32, tag="ps")
            nc.tensor.matmul(ps[:, 0, :], kT[0:64, p, bass.ts(mb, 128)],
                             qT[0:64, p, :],
                             start=True, stop=True, tile_position=(0, 0))
            nc.tensor.matmul(ps[:, 1, :], kT[64:128, p, bass.ts(mb, 128)],
                             qT[64:128, p, :],
                             start=True, stop=True, tile_position=(64, 0))
            e = WK.tile([128, 2, NB], BF16, tag="e")
            nc.scalar.activation(e[:], ps[:], AF.Exp, scale=SCALE)
            for j in range(2):
                nc.tensor.matmul(pos[j][:, :], vA[:, mb, 2 * p + j, :],
                                 e[:, j, :], start=(mb == 0), stop=(mb == N_MB - 1),
                                 skip_group_check=True)
        # normalize: out[dh, n] *= 1/denom[n], per head
        for j in range(2):
            po = pos[j]
            den = WK.tile([1, NB], BF16, tag="den")
            nc.vector.tensor_copy(den[:], po[DH:DH + 1, :])
            bc = PJ.tile([128, NB], F32, tag="pj")
            nc.tensor.matmul(bc[0:DH, :], ones64[:], den[:],
                             start=True, stop=True)
            bcs = WK.tile([64, NB], F32, tag="bcs")
            nc.vector.reciprocal(bcs[:], bc[0:DH, :])
            if j == 0:
                # even head of the pair lands on partitions 0:64 directly
                nc.vector.tensor_mul(out=oT[0:64, p, :],
                                     in0=po[0:DH, :], in1=bcs[:])
            else:
                # odd head: normalize to a temp, shift to partitions 64:128
                # via identity matmul (col tile_position), copy back aligned
                o4h = WK.tile([64, NB], BF16, tag="o4h")
                nc.vector.tensor_mul(out=o4h[:], in0=po[0:DH, :], in1=bcs[:])
                psh = PJ.tile([128, NB], F32, tag="pj")
                nc.tensor.matmul(psh[64:128, :], ident[:], o4h[:],
                                 start=True, stop=True, tile_position=(0, 64))
                nc.vector.tensor_copy(oT[64:128, p, :], psh[64:128, :])

    # ---- to_out (full Wo, disjoint output rows) ----
    for db in range(8):
        pf = PJ.tile([128, NB], F32, tag="pj")
        for kb in range(8):
            nc.tensor.matmul(pf[:, :], wo[:, kb, bass.ts(db, 128)],
                             oT[:, kb, :], start=(kb == 0), stop=(kb == 7))
        f = WK.tile([128, NB], BF16, tag="fout")
        nc.any.tensor_copy(f[:], pf[:, :])
        nc.sync.dma_start(d["outT"][bass.ts(db, 128), :], f[:])

    ctx.close()


def build_nc(wqT, wkT, wvT, woT, gather=True):
    """wqT/wkT/wvT: [D, INNER] bf16 (W.T); woT: [INNER, D] bf16 (Wo.T)."""
    from concourse import bacc
    nc = bacc.Bacc(None, target_bir_lowering=False, num_devices=8)
    cs_rows = NQ if gather else M
    d = {
        "xs": nc.dram_tensor("xs", [NQ, D], BF16, kind="ExternalInput"),
        "cs": nc.dram_tensor("cs", [cs_rows, D], BF16, kind="ExternalInput"),
        "abT": nc.dram_tensor("abT", [D, 2 * R], BF16, kind="ExternalInput"),
        "bkT0": nc.dram_tensor("bkT0", [2 * R, INNER], BF16, kind="ExternalInput"),
        "b0vT": nc.dram_tensor("b0vT", [2 * R, INNER], BF16, kind="ExternalInput"),
        "outT": nc.dram_tensor("outT", [D, NQ], BF16, kind="ExternalOutput"),
        "wqT": nc.inline_tensor(wqT, name="wqT"),
        "wkT": nc.inline_tensor(wkT, name="wkT"),
        "wvT": nc.inline_tensor(wvT, name="wvT"),
        "woT": nc.inline_tensor(woT, name="woT"),
    }
    with tile.TileContext(nc) as tc:
        _emit(tc, nc, d, gather)
    nc.compile()
    return nc


# --------------------------------------------------------------------------
# host side: per-call input packing
# --------------------------------------------------------------------------

def pack_inputs(x, context, task_idx, Ak, Bk, Av, Bv, gather=True):
    """Concatenated (along axis 0, device order) per-core input arrays."""
    bf = ml_dtypes.bfloat16
    xs = np.ascontiguousarray(np.asarray(x, np.float32)).astype(bf)
    cs = np.ascontiguousarray(np.asarray(context, np.float32)).astype(bf)
    xs_cat = xs.reshape(B * N, D)            # dev order == row order
    if gather:
        cs_cat = cs.reshape(B * M, D)
    else:
        cs_cat = np.concatenate([cs[dev // 4] for dev in range(8)], axis=0)
    abT, bkT0, b0vT = [], [], []
    z = np.zeros((R, INNER), np.float32)
    for b in range(B):
        t = int(task_idx[b])
        a = np.concatenate([Ak[t].T, Av[t].T], axis=1).astype(bf)      # [D, 32]
        bk0 = np.concatenate([Bk[t].T, z], axis=0).astype(bf)          # [32, INNER]
        b0v = np.concatenate([z, Bv[t].T], axis=0).astype(bf)
        abT += [a] * 4
        bkT0 += [bk0] * 4
        b0vT += [b0v] * 4
    return {
        "xs": xs_cat,
        "cs": cs_cat,
        "abT": np.concatenate(abT, axis=0),
        "bkT0": np.concatenate(bkT0, axis=0),
        "b0vT": np.concatenate(b0vT, axis=0),
    }


def unpack_output(out_cat, bo):
    """out_cat: [8*D, NQ] bf16 (concat of per-core outT). -> [B, N, D] f32."""
    o = np.asarray(out_cat).astype(np.float32).reshape(8, D, NQ)
    o = o.transpose(0, 2, 1).reshape(B, N, D)
    return o + np.asarray(bo, np.float32)


# --------------------------------------------------------------------------
# dispatch: cached jitted executable over the 8 cores
# --------------------------------------------------------------------------

def _weights_key(wqT, wkT, wvT, woT):
    h = hashlib.md5()
    for a in (wqT, wkT, wvT, woT):
        h.update(a.tobytes())
    return h.hexdigest()


def _get_state(wqT, wkT, wvT, woT, gather=True):
    key = (_weights_key(wqT, wkT, wvT, woT), gather)
    if key in _STATE:
        return _STATE[key]

    import jax
    import jax.numpy as jnp
    from jax.sharding import Mesh, PartitionSpec, NamedSharding
    try:
        from jax import shard_map
        def _smap(f, mesh, in_specs, out_specs):
            return shard_map(f, mesh=mesh, in_specs=in_specs,
                             out_specs=out_specs, check_vma=False)
    except ImportError:
        from jax.experimental.shard_map import shard_map
        def _smap(f, mesh, in_specs, out_specs):
            return shard_map(f, mesh=mesh, in_specs=in_specs,
                             out_specs=out_specs, check_rep=False)
    import concourse.bass2jax as b2j

    nc = build_nc(wqT, wkT, wvT, woT, gather=gather)
    b2j.install_neuronx_cc_hook()

    partition_name = nc.partition_id_tensor.name if nc.partition_id_tensor else None
    in_names, out_names, out_avals = [], [], []
    for alloc in nc.m.functions[0].allocations:
        if not isinstance(alloc, mybir.MemoryLocationSet):
            continue
        name = alloc.memorylocations[0].name
        if alloc.kind == "ExternalInput":
            if name != partition_name:
                in_names.append(name)
        elif alloc.kind == "ExternalOutput":
            shape = tuple(alloc.tensor_shape)
            dtype = mybir.dt.np(alloc.dtype)
            out_names.append(name)
            out_avals.append(jax.core.ShapedArray(shape, dtype))
    n_params = len(in_names)
    n_outs = len(out_names)
    all_in_names = list(in_names) + list(out_names)
    if partition_name is not None:
        all_in_names.append(partition_name)
    donate = tuple(range(n_params, n_params + n_outs))

    def _body(*args):
        operands = list(args)
        if partition_name is not None:
            operands.append(b2j.partition_id_tensor())
        outs = b2j._bass_exec_p.bind(
            *operands,
            out_avals=tuple(out_avals),
            in_names=tuple(all_in_names),
            out_names=tuple(out_names),
            lowering_input_output_aliases=(),
            sim_require_finite=True,
            sim_require_nnan=True,
            nc=nc,
        )
        return tuple(outs)

    devices = jax.devices()[:8]
    mesh = Mesh(np.asarray(devices), ("core",))
    spec = PartitionSpec("core")
    sharded = jax.jit(
        _smap(_body, mesh, (spec,) * (n_params + n_outs), (spec,) * n_outs),
        donate_argnums=donate, keep_unused=True,
    )
    zero_shardings = [NamedSharding(mesh, spec)] * n_outs
    zero_shapes = [(8 * a.shape[0], *a.shape[1:]) for a in out_avals]
    zero_dtypes = [a.dtype for a in out_avals]

    def _mk_zeros():
        return tuple(jnp.zeros(s, d) for s, d in zip(zero_shapes, zero_dtypes))
    zeros_maker = jax.jit(_mk_zeros, out_shardings=tuple(zero_shardings))

    st = {
        "nc": nc, "sharded": sharded, "zeros_maker": zeros_maker,
        "in_names": in_names, "out_names": out_names, "gather": gather,
    }
    _STATE[key] = st
    return st


def _run(st, packed):
    import jax
    ins = [packed[name] for name in st["in_names"]]
    zeros = st["zeros_maker"]()
    out_arrs = st["sharded"](*ins, *zeros)
    return np.asarray(out_arrs[0])


def kernel(x, context, mask, task_idx, Wq, Wk, Wv, Ak, Bk, Av, Bv, Wo, bo,
           _gather=True):
    # mask is all-ones per the input spec; softmax ignores it.
    bf = ml_dtypes.bfloat16
    wqT = np.ascontiguousarray(np.asarray(Wq, np.float32).T).astype(bf)
    wkT = np.ascontiguousarray(np.asarray(Wk, np.float32).T).astype(bf)
    wvT = np.ascontiguousarray(np.asarray(Wv, np.float32).T).astype(bf)
    woT = np.ascontiguousarray(np.asarray(Wo, np.float32).T).astype(bf)
    st = _get_state(wqT, wkT, wvT, woT, gather=_gather)
    packed = pack_inputs(x, context, np.asarray(task_idx),
                         np.asarray(Ak), np.asarray(Bk),
                         np.asarray(Av), np.asarray(Bv), gather=_gather)
    out_cat = _run(st, packed)
    return unpack_output(out_cat, bo)
